# revision 10
# baseline (speedup 1.0000x reference)
"""Cross-attention without softmax on 8 trn2 NeuronCores.

Reference computes out = (X Wq^T) (C Wk^T)^T (C Wv^T) * D^-0.5 per batch.
With no softmax the product reassociates:

    out_b = X_b @ A_b,   A_b = scale * Wq^T Wk (C_b^T C_b) Wv^T

which collapses the O(Sq*Skv*D) attention into two O(S*D^2) matmuls plus
a few 128x128 products. Sharding: batch (4) x query-half (2) -> 8 cores;
each core redundantly computes its batch's G = C^T C (no collectives).

v2: the host supplies X^T (and receives out^T), so the kernel never
transposes on-chip: out^T = A^T @ X^T with A as the PE-stationary
operand and X^T streamed 512 columns at a time. This removes all 16 PE
transposes, the identity matrix, and 4 DVE copies from the old design.
ctx is loaded in 6 chunks (small first chunk -> early G start, small
last chunk -> short G tail); PSUM->SBUF casts are spread over
Vector/Scalar/GpSimd and out stores over Sync/Tensor so no single
engine serializes the output phase.

I/O is bf16 (halves HBM traffic); accumulation stays fp32 in PSUM.
ctx row-tiles use a permuted grouping (partition p holds DRAM rows
p*r+j) so every DMA moves >=1KB contiguous per partition; G's row-sum
is invariant to that permutation.
"""

import os
import sys
import types

import numpy as np

_TRN_REPO = "/opt/trn_rl_repo"
if _TRN_REPO not in sys.path and not any("trn_rl_repo" in p for p in sys.path):
    sys.path.insert(0, _TRN_REPO)

import ml_dtypes  # noqa: E402

import concourse.bass as bass  # noqa: E402
import concourse.mybir as mybir  # noqa: E402
from concourse import bacc  # noqa: E402
from concourse.bass_utils import run_bass_kernel_spmd  # noqa: E402

B, SQ, SKV, D = 4, 4096, 4096, 128
N_CORES = 8
SQ_SHARD = SQ // (N_CORES // B)  # 2048
SCALE = float(D) ** -0.5
F32 = mybir.dt.float32
BF16 = mybir.dt.bfloat16

# ctx chunk sizes in rows: small first chunk for an early G start, small
# last chunk for a short post-load G tail. Must sum to SKV.
CTX_CHUNKS = [512, 512, 1024, 1024, 512, 512]
assert sum(CTX_CHUNKS) == SKV

_CACHE: dict = {}


def _install_axon_ntff_shim():
    try:
        import antenv.axon_hooks  # noqa: F401

        return
    except Exception:
        pass
    try:
        from trn_agent_boot.trn_boot import _ntff_profile_via_ctypes

        import antenv

        hook = _ntff_profile_via_ctypes("/opt/axon/libaxon_pjrt.so")
        mod = types.ModuleType("antenv.axon_hooks")
        mod._hook = hook
        mod.get_axon_ntff_profile_hook = lambda: mod._hook

        def _set(h):
            mod._hook = h

        mod.set_axon_ntff_profile_hook = _set
        antenv.axon_hooks = mod
        sys.modules["antenv.axon_hooks"] = mod
    except Exception:
        pass

    try:
        import concourse.bass_utils as bu

        bu.upload_artifacts = lambda tmpdir: f"file://{tmpdir}"
    except Exception:
        pass


def build_v2():
    """Per-core inputs: xt = X_shard^T [128, 2048], ctx [4096, 128],
    w = [wq*scale | wk | wv^T] packed [128, 384]; output outt = out^T
    [128, 2048]. All bf16.

    Cumulative PE schedule (s_pe value after the op):
      UT=1, G chunks 2..33, P=34, A=35, outT mm 36..39.
    DVE chain copies on s_dve: ut=1 gs=2 ps=3 a=4.
    Out casts: s_o[k] (vector: 0,3; scalar: 1; gpsimd: 2).
    PSUM banks: b0=G b1=UT b2=P b3=A b4..7=outT chunks.
    """
    from contextlib import ExitStack

    cdt = BF16
    nc = bacc.Bacc(None, target_bir_lowering=False, debug=False)
    xt_ext = nc.declare_dram_parameter("xt", [D, SQ_SHARD], cdt, isOutput=False)
    c_ext = nc.declare_dram_parameter("ctx", [SKV, D], cdt, isOutput=False)
    w_ext = nc.declare_dram_parameter("w", [D, 3 * D], cdt, isOutput=False)
    outt_ext = nc.declare_dram_parameter(
        "outt", [D, SQ_SHARD], cdt, isOutput=True
    )

    ncc = len(CTX_CHUNKS)
    offs = [sum(CTX_CHUNKS[:i]) for i in range(ncc)]
    rpp = [n // 128 for n in CTX_CHUNKS]  # rows per partition per chunk
    ctx_view = [
        c_ext[offs[i] : offs[i] + CTX_CHUNKS[i], :].rearrange(
            "(p r) d -> p r d", p=128
        )
        for i in range(ncc)
    ]

    es = ExitStack()
    _n = [0]

    def sb(shape, dt, name=None):
        _n[0] += 1
        return es.enter_context(nc.sbuf_tensor(name or f"sb{_n[0]}", shape, dt))

    def pst(shape, dt, name=None):
        _n[0] += 1
        return es.enter_context(nc.psum_tensor(name or f"ps{_n[0]}", shape, dt))

    def sem(name):
        return es.enter_context(nc.semaphore(name))

    with es:
        w_sb = sb([D, 3 * D], cdt, "w_sb")
        cc = [sb([128, rpp[i], D], cdt, f"cc{i}") for i in range(ncc)]
        xt_sb = sb([D, SQ_SHARD], cdt, "xt_sb")
        ut_sb = sb([D, D], cdt, "ut_sb")
        gs = sb([D, D], cdt, "gs")
        ps_sb = sb([D, D], cdt, "ps_sb")
        a_sb = sb([D, D], cdt, "a_sb")
        o_sb = [sb([128, 512], cdt, f"o_sb{k}") for k in range(4)]

        g_ps = pst([128, 512], F32)  # b0 (use [:, :128])
        ut_ps = pst([128, 512], F32)  # b1
        p_ps = pst([128, 512], F32)  # b2
        a_ps = pst([128, 512], F32)  # b3
        o_ps = [pst([128, 512], F32) for _ in range(4)]  # b4..b7

        s_w = sem("s_w")
        s_x = sem("s_x")
        s_c = [sem(f"s_c{i}") for i in range(ncc)]
        s_pe = sem("s_pe")
        s_dve = sem("s_dve")
        s_o = [sem(f"s_o{k}") for k in range(4)]
        s_st = sem("s_st")

        # s_pe indices
        pe_ut = 1
        pe_g = [2 + sum(rpp[:i]) for i in range(ncc)]  # first mm of chunk i
        pe_g_done = 1 + sum(rpp)  # 33
        pe_p = pe_g_done + 1  # 34
        pe_a = pe_p + 1  # 35
        pe_o = [pe_a + 1 + k for k in range(4)]  # 36..39

        with nc.Block() as block:

            @block.sync
            def _(sync):
                nc.sync.dma_start(cc[0][:], ctx_view[0]).then_inc(s_c[0], 16)
                nc.sync.dma_start(cc[2][:], ctx_view[2]).then_inc(s_c[2], 16)
                nc.sync.dma_start(cc[4][:], ctx_view[4]).then_inc(s_c[4], 16)
                nc.sync.wait_ge(s_o[0], 1)
                nc.sync.dma_start(outt_ext[:, 0:512], o_sb[0][:]).then_inc(
                    s_st, 16
                )
                nc.sync.wait_ge(s_o[2], 1)
                nc.sync.dma_start(
                    outt_ext[:, 1024:1536], o_sb[2][:]
                ).then_inc(s_st, 16)
                nc.sync.wait_ge(s_st, 64)

            @block.scalar
            def _(sc):
                nc.scalar.dma_start(cc[1][:], ctx_view[1]).then_inc(s_c[1], 16)
                nc.scalar.dma_start(cc[3][:], ctx_view[3]).then_inc(s_c[3], 16)
                nc.scalar.dma_start(cc[5][:], ctx_view[5]).then_inc(s_c[5], 16)
                nc.scalar.dma_start(xt_sb[:], xt_ext[:]).then_inc(s_x, 16)
                nc.scalar.wait_ge(s_pe, pe_o[1])
                nc.scalar.copy(o_sb[1][:], o_ps[1][:]).then_inc(s_o[1], 1)
                nc.scalar.dma_start(
                    outt_ext[:, 512:1024], o_sb[1][:]
                ).then_inc(s_st, 16)
                nc.scalar.wait_ge(s_pe, pe_o[3])
                nc.scalar.copy(o_sb[3][:], o_ps[3][:]).then_inc(s_o[3], 1)
                nc.scalar.dma_start(
                    outt_ext[:, 1536:2048], o_sb[3][:]
                ).then_inc(s_st, 16)
                nc.scalar.wait_ge(s_st, 64)

            @block.gpsimd
            def _(gp):
                nc.gpsimd.dma_start(w_sb[:], w_ext[:]).then_inc(s_w, 16)


            @block.tensor
            def _(te):
                nc.tensor.wait_ge(s_w, 16)
                # UT = Wk^T (scale*Wq)
                nc.tensor.matmul(
                    ut_ps[:, :128],
                    w_sb[:, 128:256],
                    w_sb[:, 0:128],
                    start=True,
                    stop=True,
                ).then_inc(s_pe, 1)
                # G = C^T C accumulated over all row-tiles
                for c in range(ncc):
                    nc.tensor.wait_ge(s_c[c], 16)
                    for j in range(rpp[c]):
                        nc.tensor.matmul(
                            g_ps[:, :128],
                            cc[c][:, j, :],
                            cc[c][:, j, :],
                            start=(c == 0 and j == 0),
                            stop=(c == ncc - 1 and j == rpp[c] - 1),
                        ).then_inc(s_pe, 1)
                # P = G Wv^T
                nc.tensor.wait_ge(s_dve, 2)
                nc.tensor.matmul(
                    p_ps[:, :128],
                    gs[:],
                    w_sb[:, 256:384],
                    start=True,
                    stop=True,
                ).then_inc(s_pe, 1)
                # A = U P  (lhsT = U^T)
                nc.tensor.wait_ge(s_dve, 3)
                nc.tensor.matmul(
                    a_ps[:, :128], ut_sb[:], ps_sb[:], start=True, stop=True
                ).then_inc(s_pe, 1)
                # out^T = A^T X^T in 4 chunks of 512 query columns
                nc.tensor.wait_ge(s_dve, 4)
                nc.tensor.wait_ge(s_x, 16)
                for k in range(4):
                    nc.tensor.matmul(
                        o_ps[k][:],
                        a_sb[:],
                        xt_sb[:, 512 * k : 512 * (k + 1)],
                        start=True,
                        stop=True,
                    ).then_inc(s_pe, 1)


            @block.vector
            def _(ve):
                nc.vector.wait_ge(s_pe, pe_ut)
                nc.vector.tensor_copy(ut_sb[:], ut_ps[:, :128]).then_inc(
                    s_dve, 1
                )
                nc.vector.wait_ge(s_pe, pe_g_done)
                nc.vector.tensor_copy(gs[:], g_ps[:, :128]).then_inc(s_dve, 1)
                nc.vector.wait_ge(s_pe, pe_p)
                nc.vector.tensor_copy(ps_sb[:], p_ps[:, :128]).then_inc(
                    s_dve, 1
                )
                nc.vector.wait_ge(s_pe, pe_a)
                nc.vector.tensor_copy(a_sb[:], a_ps[:, :128]).then_inc(
                    s_dve, 1
                )
                nc.vector.wait_ge(s_pe, pe_o[0])
                nc.vector.tensor_copy(o_sb[0][:], o_ps[0][:]).then_inc(
                    s_o[0], 1
                )
                nc.vector.wait_ge(s_pe, pe_o[2])
                nc.vector.tensor_copy(o_sb[2][:], o_ps[2][:]).then_inc(
                    s_o[2], 1
                )

    nc.compile()
    return nc


def build():
    return build_v2()


def _get_nc():
    if "nc" not in _CACHE:
        _CACHE["nc"] = build()
    return _CACHE["nc"]


def _run(inputs: dict, trace: bool = False, **kw):
    np_dt = ml_dtypes.bfloat16
    context = np.ascontiguousarray(inputs["context"]).astype(np_dt)
    Wq = np.asarray(inputs["Wq"], dtype=np.float32) * SCALE
    Wk = np.asarray(inputs["Wk"], dtype=np.float32)
    Wvt = np.asarray(inputs["Wv"], dtype=np.float32).T
    w_pack = np.ascontiguousarray(
        np.concatenate([Wq, Wk, Wvt], axis=1)
    ).astype(np_dt)
    X = np.asarray(inputs["X"], dtype=np.float32)

    in_maps = []
    for c in range(N_CORES):
        b, h = divmod(c, 2)
        xt = np.ascontiguousarray(
            X[b, h * SQ_SHARD : (h + 1) * SQ_SHARD, :].T
        ).astype(np_dt)
        in_maps.append({"xt": xt, "ctx": context[b], "w": w_pack})

    nc = _get_nc()
    res = run_bass_kernel_spmd(
        nc, in_maps, core_ids=list(range(N_CORES)), trace=trace, **kw
    )
    out = np.empty((B, SQ, D), dtype=np.float32)
    for c in range(N_CORES):
        b, h = divmod(c, 2)
        out[b, h * SQ_SHARD : (h + 1) * SQ_SHARD, :] = (
            res.results[c]["outt"].astype(np.float32).T
        )
    return out, res


def kernel(**inputs: np.ndarray) -> np.ndarray:
    if os.environ.get("BASS_TRACE"):
        _install_axon_ntff_shim()
    try:
        out, _ = _run(inputs, trace=False)
    except Exception:
        # transient NRT device errors have been observed once across many
        # runs; one retry on a fresh execution
        out, _ = _run(inputs, trace=False)
    return out


if __name__ == "__main__":
    rng = np.random.default_rng(0)
    ins = {
        "context": rng.standard_normal((B, SKV, D)).astype(np.float32),
        "X": rng.standard_normal((B, SQ, D)).astype(np.float32),
        "Wq": (rng.standard_normal((D, D)) / np.sqrt(D)).astype(np.float32),
        "Wk": (rng.standard_normal((D, D)) / np.sqrt(D)).astype(np.float32),
        "Wv": (rng.standard_normal((D, D)) / np.sqrt(D)).astype(np.float32),
    }
    got = kernel(**ins)
    q = ins["X"] @ ins["Wq"].T
    k = ins["context"] @ ins["Wk"].T
    v = ins["context"] @ ins["Wv"].T
    w = np.einsum("bse,bte->bst", q, k) * SCALE
    want = np.einsum("bst,bte->bse", w, v)
    rel = np.linalg.norm(got - want) / np.linalg.norm(want)
    print("rel err vs numpy:", rel)
